# revision 20
# baseline (speedup 1.0000x reference)
"""BagNet Trainium2 Bass kernel, 8-core data parallel — v4/v5 layouts.

Cost model: matmul time = N (streamed output columns) x 0.42ns, independent
of K and M.  v3 streamed ~1.13M rows; v4/v5 restructures to ~716K:

  - conv1 ki-fold (v4): input stored as 3 y-shifted replicas stacked on the
    K axis (xt3 [84=(ky,x), p, y']), so each x-block h needs ONE K=84 matmul
    instead of 3 K=28 matmuls.  39 -> 13 matmuls/group (146K -> 49K rows).
  - conv2 parity-merge (v4): M=128=(xo-parity, co) instead of M=64=co.  Each
    matmul's lhsT covers 3 of the 9 taps (zero quadrant = 1/4 waste); 2
    matmuls per ki x 3 ki = 6 K=128/M=128 matmuls per patch cover everything
    (was 12 at M=64).  497K -> 249K rows.
  - conv3 D-repair (v5): the 6 per-patch K=64 single-tap matmuls are
    re-paired into K=128 by building y-shifted copies (D_e/D_o) of the A2
    dxo-planes via SBUF->SBUF DMA.  12 -> 10 matmuls/patch-pair
    (418K -> 348K rows).

Evac (PSUM->SBUF relu+bias) on alternating Scalar/Vector writes contiguous
destinations; wf streamed + reordered host-side to h3's (par, xg, yo)
s-order (unchanged from v3).
"""
import os
import numpy as np
import ml_dtypes
from contextlib import ExitStack

import concourse.bass as bass
import concourse.tile as tile
from concourse import bacc, mybir
from concourse.bass_utils import run_bass_kernel_spmd

LAST_EXEC_TIME_NS = None
LAST_PROFILE = None

EPS = 1e-5
NCORES = 8
BC = 16          # images per core
P = BC * 9       # patches per core
G = 16           # patches per pipeline group
NG = P // G
CH_S = int(os.environ.get("KN_CHS", "22"))   # wf s-slices per DMA chunk
NCH = 484 // CH_S
assert CH_S * NCH == 484
DT = mybir.dt.bfloat16
NPDT = ml_dtypes.bfloat16
F32 = mybir.dt.float32
RELU = mybir.ActivationFunctionType.Relu
IDENT = mybir.ActivationFunctionType.Identity
ADD = mybir.AluOpType.add
MAX = mybir.AluOpType.max

EV1 = os.environ.get("KN_EV1", "sv")
EV2 = os.environ.get("KN_EV2", "sv")
EV3 = os.environ.get("KN_EV3", "sv")
PSBUFS = int(os.environ.get("KN_PSBUFS", "7"))
WFB = int(os.environ.get("KN_WFB", "3"))
USE_D = bool(int(os.environ.get("KN_D", "0")))   # conv3 D-repair vs singles
WF2R = bool(int(os.environ.get("KN_WF2R", "0")))  # wf on both HW-DGE rings
PH = frozenset(os.environ.get("KN_PH", "c1,c2,c3,fc").split(","))


def build_program(repeat=1):
    nc = bacc.Bacc("TRN2", target_bir_lowering=False, debug=False)

    xt_d = nc.dram_tensor("xt3", [84, P * 26], DT, kind="ExternalInput")
    w1_d = nc.dram_tensor("w1k", [84, 13 * 128], DT, kind="ExternalInput")
    w2_d = nc.dram_tensor("w2m", [128, 6 * 128], DT, kind="ExternalInput")
    w3p_d = nc.dram_tensor("w3p", [128, 6 * 128], DT, kind="ExternalInput")
    w3s_d = nc.dram_tensor("w3s", [128, 3 * 128], DT, kind="ExternalInput")
    w3d_d = nc.dram_tensor("w3d", [128, 4 * 128], DT, kind="ExternalInput")
    b1_d = nc.dram_tensor("b1v", [128, 1], F32, kind="ExternalInput")
    b2_d = nc.dram_tensor("b2v", [128, 1], F32, kind="ExternalInput")
    b3_d = nc.dram_tensor("b3v", [128, 1], F32, kind="ExternalInput")
    bf_d = nc.dram_tensor("bfv", [100, 1], F32, kind="ExternalInput")
    # wfq laid out so each FC chunk is one fully-contiguous DRAM block:
    # row block c*128..(c+1)*128 holds chunk c as [128, CH_S*100]
    wf_d = nc.dram_tensor("wfq", [NCH * 128, CH_S * 100], DT,
                          kind="ExternalInput")
    out_d = nc.dram_tensor("out", [100, P], F32, kind="ExternalOutput")

    def evac(mode, idx, dst, src, bias):
        """relu(src + bias) -> dst, engine rotation string over
        ScalarE ('s'), VectorE ('v'), GpSimd/Pool ('g')."""
        eng = mode[idx % len(mode)]
        if eng == "s":
            nc.scalar.activation(dst, src, RELU, bias=bias, scale=1.0)
        elif eng == "g":
            nc.gpsimd.tensor_scalar(dst, src, bias, 0.0, ADD, MAX)
        else:
            nc.vector.tensor_scalar(dst, src, bias, 0.0, ADD, MAX)

    with tile.TileContext(nc) as tc, ExitStack() as ctx:
        const = ctx.enter_context(tc.tile_pool(name="const", bufs=1))
        a1p = ctx.enter_context(
            tc.tile_pool(name="a1", bufs=int(os.environ.get("KN_A1B", "1"))))
        a2p = ctx.enter_context(tc.tile_pool(name="a2", bufs=1))
        dp = ctx.enter_context(tc.tile_pool(name="dd", bufs=1))
        h3p = ctx.enter_context(tc.tile_pool(name="h3", bufs=1))
        wfp = ctx.enter_context(tc.tile_pool(name="wf", bufs=WFB))
        ps = ctx.enter_context(tc.tile_pool(name="ps", bufs=PSBUFS, space="PSUM"))
        fcp = ctx.enter_context(tc.tile_pool(name="fcps", bufs=1, space="PSUM"))

        xt3 = const.tile([84, P, 26], DT)
        nc.sync.dma_start(xt3[:], xt_d[:].rearrange("a (p y) -> a p y", y=26))
        w1k = const.tile([84, 13 * 128], DT)
        nc.sync.dma_start(w1k[:], w1_d[:])
        w2m = const.tile([128, 6 * 128], DT)
        nc.sync.dma_start(w2m[:], w2_d[:])
        w3pr = const.tile([128, 6 * 128], DT)
        nc.sync.dma_start(w3pr[:], w3p_d[:])
        w3sg = const.tile([128, 3 * 128], DT)
        nc.sync.dma_start(w3sg[:], w3s_d[:])
        w3d = const.tile([128, 4 * 128], DT)
        nc.sync.dma_start(w3d[:], w3d_d[:])
        b1t = const.tile([128, 1], F32)
        nc.sync.dma_start(b1t[:], b1_d[:])
        b2t = const.tile([128, 1], F32)
        nc.sync.dma_start(b2t[:], b2_d[:])
        b3t = const.tile([128, 1], F32)
        nc.sync.dma_start(b3t[:], b3_d[:])
        bft = const.tile([100, 1], F32)
        nc.sync.dma_start(bft[:], bf_d[:])

        # h3 is s-major so the FC matmul rhs columns are CONTIGUOUS
        # (a patch-major h3 makes the PE stream 2-byte strided columns,
        # which measured ~4x slower than contiguous). The conv3 evacs pay
        # with strided writes instead, hidden under the tensor-bound convs.
        h3 = h3p.tile([128, 484, P], DT)
        fc_psum = fcp.tile([128, P], F32)

        rep_ctx = tc.For_i(0, repeat, 1) if repeat > 1 else None
        if rep_ctx is not None:
            rep_ctx.__enter__()
        # pre-issue the first WFB wf chunks so they stream during the convs;
        # alternate the two physical HW-DGE rings (SP / Activation issued)
        wf_eng = [nc.sync, nc.scalar] if WF2R else [nc.sync, nc.sync]
        wf_pre = []
        for c in range(min(WFB, NCH) if "fc" in PH else 0):
            wfb = wfp.tile([128, CH_S * 100], DT)
            wf_eng[c % 2].dma_start(wfb[:], wf_d[c * 128:(c + 1) * 128, :])
            wf_pre.append(wfb)
        for g in range(NG if "c1" in PH else 0):
            A1x = a1p.tile([128, 13, G, 26], DT)
            A2 = a2p.tile([128, G, 12, 24], DT)
            if USE_D:
                D_e = dp.tile([128, G, 12, 24], DT)
                D_o = dp.tile([128, G, 12, 24], DT)
                A2f = A2[:, :, :, :].rearrange("q g n y -> q (g n y)")
                D_ef = D_e[:, :, :, :].rearrange("q g n y -> q (g n y)")
                D_of = D_o[:, :, :, :].rearrange("q g n y -> q (g n y)")

            # ---- conv1: K=84=(ky,x) ki-folded; 13 matmuls of N=416 ----
            for h in range(13):
                c1 = ps.tile([128, G * 26], F32, tag="cps")
                nc.tensor.matmul(
                    c1[:, :], w1k[0:84, h * 128:(h + 1) * 128],
                    xt3[0:84, g * G:(g + 1) * G, 0:26],
                    start=True, stop=True)
                evac(EV1, h, A1x[:, h, :, :], c1[:, :], b1t[:, :])

            # ---- conv2: parity-merged M=128=(par,co); 6 matmuls/patch ----
            for pl in range(G if "c2" in PH else 0):
                c2 = ps.tile([128, 12, 24], F32, tag="cps")
                for ki in range(3):
                    nc.tensor.matmul(
                        c2[:], w2m[:, (2 * ki) * 128:(2 * ki + 1) * 128],
                        A1x[:, 0:12, pl, ki:ki + 24],
                        start=(ki == 0), stop=False)
                    nc.tensor.matmul(
                        c2[:], w2m[:, (2 * ki + 1) * 128:(2 * ki + 2) * 128],
                        A1x[:, 1:13, pl, ki:ki + 24],
                        start=False, stop=(ki == 2))
                evac(EV2, pl, A2[:, pl, :, :], c2[:], b2t[:, :])
                if USE_D and pl % 2 == 1 and "c3" in PH:
                    # flat-offset y-shift copies of the A2 dxo-planes for this
                    # patch pair: band0 = plane, band1 = plane shifted +1 elem
                    # (wrapped slots land at y=23, never read by the matmuls)
                    b0 = (pl - 1) * 288
                    nc.sync.dma_start(
                        D_ef[0:64, b0:b0 + 576], A2f[0:64, b0:b0 + 576])
                    nc.sync.dma_start(
                        D_ef[64:128, b0:b0 + 575], A2f[0:64, b0 + 1:b0 + 576])
                    nc.sync.dma_start(
                        D_of[0:64, b0:b0 + 576], A2f[64:128, b0:b0 + 576])
                    nc.sync.dma_start(
                        D_of[64:128, b0:b0 + 575], A2f[64:128, b0 + 1:b0 + 576])

            # ---- conv3: pairs + (D-repair | K=64 singles) ----
            if "c3" not in PH:
                continue
            if USE_D:
                for pq in range(G // 2):
                    pl = 2 * pq
                    pv = slice(pl, pl + 2)
                    c3e = ps.tile([128, 2, 11, 22], F32, tag="cps")
                    c3o = ps.tile([128, 2, 11, 22], F32, tag="cps")
                    for ki in range(3):
                        yv = slice(ki, ki + 22)
                        nc.tensor.matmul(
                            c3e[:], w3pr[:, ki * 128:(ki + 1) * 128],
                            A2[:, pv, 0:11, yv], start=(ki == 0), stop=False)
                        nc.tensor.matmul(
                            c3o[:], w3pr[:, (3 + ki) * 128:(4 + ki) * 128],
                            A2[:, pv, 1:12, yv], start=(ki == 0), stop=False)
                    nc.tensor.matmul(  # taps (kh 0+1, kw2) via D_e bands
                        c3e[:], w3d[:, 0:128],
                        D_e[:, pv, 1:12, 0:22], start=False, stop=False)
                    nc.tensor.matmul(  # tap (kh2, kw2) direct from A2
                        c3e[:], w3d[0:64, 128:256],
                        A2[0:64, pv, 1:12, 2:24], start=False, stop=True)
                    nc.tensor.matmul(  # taps (kh 0+1, kw0) via D_o bands
                        c3o[:], w3d[:, 2 * 128:3 * 128],
                        D_o[:, pv, 0:11, 0:22], start=False, stop=False)
                    nc.tensor.matmul(  # tap (kh2, kw0) direct from A2
                        c3o[:], w3d[64:128, 3 * 128:4 * 128],
                        A2[64:128, pv, 0:11, 2:24], start=False, stop=True)
                    pg = g * G + pl
                    evac(EV3, pq,
                         h3[:, 0:242, pg:pg + 2].rearrange("q s p -> q p s"),
                         c3e[:], b3t[:, :])
                    evac(EV3, pq + 1,
                         h3[:, 242:484, pg:pg + 2].rearrange("q s p -> q p s"),
                         c3o[:], b3t[:, :])
            else:
                for pq in range(G // 2):
                    pl = 2 * pq
                    pv = slice(pl, pl + 2)
                    c3e = ps.tile([128, 2, 11, 22], F32, tag="cps")
                    c3o = ps.tile([128, 2, 11, 22], F32, tag="cps")
                    for ki in range(3):
                        yv = slice(ki, ki + 22)
                        nc.tensor.matmul(
                            c3e[:], w3pr[:, ki * 128:(ki + 1) * 128],
                            A2[:, pv, 0:11, yv], start=(ki == 0), stop=False)
                        nc.tensor.matmul(
                            c3o[:], w3pr[:, (3 + ki) * 128:(4 + ki) * 128],
                            A2[:, pv, 1:12, yv], start=(ki == 0), stop=False)
                        nc.tensor.matmul(
                            c3e[:], w3sg[0:64, ki * 128:(ki + 1) * 128],
                            A2[0:64, pv, 1:12, yv], start=False, stop=(ki == 2))
                        nc.tensor.matmul(
                            c3o[:], w3sg[64:128, ki * 128:(ki + 1) * 128],
                            A2[64:128, pv, 0:11, yv], start=False, stop=(ki == 2))
                    pg = g * G + pl
                    evac(EV3, pq,
                         h3[:, 0:242, pg:pg + 2].rearrange("q s p -> q p s"),
                         c3e[:], b3t[:, :])
                    evac(EV3, pq + 1,
                         h3[:, 242:484, pg:pg + 2].rearrange("q s p -> q p s"),
                         c3o[:], b3t[:, :])

        # ---- FC: 484 accumulating K=128 matmuls, wf streamed ----
        if "fc" not in PH:
            nc.vector.memset(fc_psum[0:100, :], 0.0)
        if "fc" in PH and "c3" not in PH:
            nc.vector.memset(h3[:], 0.0)
        for c in range(NCH if "fc" in PH else 0):
            if c < len(wf_pre):
                wfb = wf_pre[c]
            else:
                wfb = wfp.tile([128, CH_S * 100], DT)
                wf_eng[c % 2].dma_start(
                    wfb[:], wf_d[c * 128:(c + 1) * 128, :])
            for sl in range(CH_S):
                s = c * CH_S + sl
                nc.tensor.matmul(
                    fc_psum[0:100, :], wfb[:, sl * 100:(sl + 1) * 100],
                    h3[:, s, :],
                    start=(s == 0), stop=(s == 483))
        outb = const.tile([100, P], F32)
        nc.scalar.activation(outb[:], fc_psum[0:100, :], IDENT,
                             bias=bft[:, :], scale=1.0)
        nc.sync.dma_start(out_d[:], outb[:])
        if rep_ctx is not None:
            rep_ctx.__exit__(None, None, None)

    nc.compile()
    return nc


def _fold_bn(w, b, g_, be, m, v):
    s = (g_ / np.sqrt(v + EPS)).astype(np.float32)
    return (w * s[:, None, None, None]).astype(np.float32), \
           (be - (m - b) * s).astype(np.float32)


def prep_shared(inputs):
    """Host-side: fold BN, build weight layouts shared by all cores."""
    f = np.float32
    w1f, b1f = _fold_bn(inputs["w1"], inputs["b1"], inputs["g1"], inputs["be1"],
                        inputs["m1"], inputs["v1"])
    w2f, b2f = _fold_bn(inputs["w2"], inputs["b2"], inputs["g2"], inputs["be2"],
                        inputs["m2"], inputs["v2"])
    w3f, b3f = _fold_bn(inputs["w3"], inputs["b3"], inputs["g3"], inputs["be3"],
                        inputs["m3"], inputs["v3"])

    # conv1 ki-folded lhsT: w1k[(ky,x), (dx,co)] per h-block
    w1r = w1f[:, 0]                      # (64, kh, kw)
    w1k = np.zeros((84, 13 * 128), f)
    for h in range(13):
        for r in range(3):
            for dx in range(2):
                for kx in range(3):
                    xx = 2 * h + dx + kx
                    w1k[28 * r + xx,
                        h * 128 + dx * 64:h * 128 + dx * 64 + 64] = w1r[:, r, kx]

    # conv2 parity-merged lhsT blocks (ki, slot): [128=(dx,ci), 128=(par,co)]
    w2m = np.zeros((128, 6 * 128), f)
    for ki in range(3):
        b0 = (2 * ki) * 128
        b1 = (2 * ki + 1) * 128
        w2m[0:64, b0:b0 + 64] = w2f[:, :, ki, 0].T       # even, kw0 @ dx0
        w2m[64:128, b0:b0 + 64] = w2f[:, :, ki, 1].T     # even, kw1 @ dx1
        w2m[64:128, b0 + 64:b0 + 128] = w2f[:, :, ki, 0].T  # odd, kw0 @ dx1
        w2m[0:64, b1:b1 + 64] = w2f[:, :, ki, 2].T       # even, kw2 @ dx0(+1)
        w2m[0:64, b1 + 64:b1 + 128] = w2f[:, :, ki, 1].T    # odd, kw1 @ dx0(+1)
        w2m[64:128, b1 + 64:b1 + 128] = w2f[:, :, ki, 2].T  # odd, kw2 @ dx1(+1)

    def tap_tiles(wt, co):  # wt (co, 64, 3, 3)
        pr = np.zeros((128, 6 * co), f)
        sg = np.zeros((128, 3 * co), f)
        for ki in range(3):
            pr[0:64, ki * co:(ki + 1) * co] = wt[:, :, ki, 0].T
            pr[64:128, ki * co:(ki + 1) * co] = wt[:, :, ki, 1].T
            pr[0:64, (3 + ki) * co:(4 + ki) * co] = wt[:, :, ki, 1].T
            pr[64:128, (3 + ki) * co:(4 + ki) * co] = wt[:, :, ki, 2].T
            sg[0:64, ki * co:(ki + 1) * co] = wt[:, :, ki, 2].T
            sg[64:128, ki * co:(ki + 1) * co] = wt[:, :, ki, 0].T
        return pr, sg

    w3p, w3s = tap_tiles(w3f, 128)

    # conv3 D-weights: blocks [De-pair, De-single, Do-pair, Do-single].
    # The Do-single reads A2[64:128] directly, so its lhsT lives at
    # partitions 64:128 (matmul requires matching base partitions).
    w3d = np.zeros((128, 4 * 128), f)
    for r in range(2):
        w3d[r * 64:(r + 1) * 64, 0:128] = w3f[:, :, r, 2].T
        w3d[r * 64:(r + 1) * 64, 2 * 128:3 * 128] = w3f[:, :, r, 0].T
    w3d[0:64, 128:256] = w3f[:, :, 2, 2].T
    w3d[64:128, 3 * 128:4 * 128] = w3f[:, :, 2, 0].T

    # wf reordered to h3's (par, xg, yo) s-ordering
    wfr = inputs["wf"].astype(f).reshape(100, 128, 22, 22)   # (out, co, yo, xo)
    wfr = wfr.transpose(0, 1, 3, 2)                          # (out, co, xo, yo)
    wfr = wfr.reshape(100, 128, 11, 2, 22).transpose(0, 1, 3, 2, 4)
    wfq = np.ascontiguousarray(
        wfr.reshape(100, 128, 484).transpose(1, 2, 0)
    ).reshape(128, 484 * 100)
    # chunk-contiguous DRAM layout: [NCH*128, CH_S*100], chunk c in rows
    # c*128..(c+1)*128
    wfq = np.ascontiguousarray(
        wfq.reshape(128, NCH, CH_S * 100).transpose(1, 0, 2)
    ).reshape(NCH * 128, CH_S * 100)

    return {
        "w1k": w1k.astype(NPDT),
        "w2m": w2m.astype(NPDT),
        "w3p": w3p.astype(NPDT), "w3s": w3s.astype(NPDT),
        "w3d": w3d.astype(NPDT),
        "b1v": np.tile(b1f, 2)[:, None].astype(f),
        "b2v": np.tile(b2f, 2)[:, None].astype(f),
        "b3v": b3f[:, None].astype(f),
        "bfv": inputs["bf"].astype(f)[:, None],
        "wfq": wfq.astype(NPDT),
    }


def prep_core(x, c):
    """Per-core input: x-pixel-on-partition patch layout, 3 y-shifted bands."""
    xs = np.asarray(x)[c * BC:(c + 1) * BC, 0].astype(np.float32)  # (16,84,84)
    arr = xs.reshape(BC, 3, 28, 3, 28).transpose(4, 0, 1, 3, 2)    # (x,b,Hb,Wb,y)
    arr = arr.reshape(28, P, 28)
    xt3 = np.zeros((84, P, 26), np.float32)
    for r in range(3):
        xt3[28 * r:28 * r + 28] = arr[:, :, r:r + 26]
    return {"xt3": xt3.reshape(84, P * 26).astype(NPDT)}


def _make_runner(nc, in_maps):
    import jax
    from jax.sharding import Mesh, PartitionSpec, NamedSharding
    from jax.experimental.shard_map import shard_map
    from concourse import mybir as _mb
    from concourse import bass2jax

    bass2jax.install_neuronx_cc_hook()
    partition_name = nc.partition_id_tensor.name if nc.partition_id_tensor else None
    in_names, out_names, out_avals, zero_outs = [], [], [], []
    for alloc in nc.m.functions[0].allocations:
        if not isinstance(alloc, _mb.MemoryLocationSet):
            continue
        name = alloc.memorylocations[0].name
        if alloc.kind == "ExternalInput":
            if name != partition_name:
                in_names.append(name)
        elif alloc.kind == "ExternalOutput":
            shape = tuple(alloc.tensor_shape)
            dtype = _mb.dt.np(alloc.dtype)
            out_names.append(name)
            out_avals.append(jax.core.ShapedArray(shape, dtype))
            zero_outs.append(np.zeros(shape, dtype))
    n_params = len(in_names)
    all_names = in_names + out_names
    if partition_name is not None:
        all_names = all_names + [partition_name]
    donate = tuple(range(n_params, n_params + len(out_names)))

    def _body(*args):
        operands = list(args)
        if partition_name is not None:
            operands.append(bass2jax.partition_id_tensor())
        outs = bass2jax._bass_exec_p.bind(
            *operands,
            out_avals=tuple(out_avals),
            in_names=tuple(all_names),
            out_names=tuple(out_names),
            lowering_input_output_aliases=(),
            sim_require_finite=True,
            sim_require_nnan=True,
            nc=nc,
        )
        return tuple(outs)

    devices = jax.devices()[:NCORES]
    mesh = Mesh(np.asarray(devices), ("core",))
    spec = NamedSharding(mesh, PartitionSpec("core"))
    sharded = jax.jit(
        shard_map(_body, mesh=mesh,
                  in_specs=(PartitionSpec("core"),) * (n_params + len(out_names)),
                  out_specs=(PartitionSpec("core"),) * len(out_names),
                  check_rep=False),
        donate_argnums=donate, keep_unused=True)

    concat_in = [
        jax.device_put(
            np.concatenate([np.asarray(in_maps[c][n]) for c in range(NCORES)],
                           axis=0), spec)
        for n in in_names
    ]

    def _zeros():
        return [jax.device_put(
            np.zeros((NCORES * z.shape[0], *z.shape[1:]), z.dtype), spec)
            for z in zero_outs]

    def dispatch():
        import time as _time
        zs = _zeros()
        jax.block_until_ready(zs)
        t0 = _time.perf_counter()
        r = sharded(*concat_in, *zs)
        jax.block_until_ready(r)
        return _time.perf_counter() - t0

    r = sharded(*concat_in, *_zeros())   # compile + warm
    jax.block_until_ready(r)
    return dispatch


def bench(inputs, iters=14, repeat=128):
    """Per-iteration HW time: interleaved dispatches of the repeat=1 and
    repeat=`repeat` programs; median difference resists the bimodal axon
    dispatch-overhead noise."""
    inputs = {k: np.asarray(v) for k, v in inputs.items()}
    shared = prep_shared(inputs)
    in_maps = [{**shared, **prep_core(inputs["x"], c)} for c in range(NCORES)]
    d1 = _make_runner(build_program(repeat=1), in_maps)
    dr = _make_runner(build_program(repeat=repeat), in_maps)
    t1s, trs = [], []
    for _ in range(iters):
        t1s.append(d1())
        trs.append(dr())
    t1s.sort()
    trs.sort()
    med1 = t1s[len(t1s) // 2]
    medr = trs[len(trs) // 2]
    med = (medr - med1) * 1e9 / (repeat - 1)
    mn = (trs[0] - t1s[0]) * 1e9 / (repeat - 1)
    print(f"[bench] med {med:.0f} ns  min {mn:.0f} ns  "
          f"t1 {[f'{t*1e3:.1f}' for t in t1s]} "
          f"tr {[f'{t*1e3:.1f}' for t in trs]}")
    return med


def kernel(**inputs):
    global LAST_EXEC_TIME_NS, LAST_PROFILE
    inputs = {k: np.asarray(v) for k, v in inputs.items()}
    shared = prep_shared(inputs)
    in_maps = [{**shared, **prep_core(inputs["x"], c)} for c in range(NCORES)]
    nc = build_program()
    trace = bool(os.environ.get("BASS_KERNEL_TRACE"))
    try:
        res = run_bass_kernel_spmd(nc, in_maps, list(range(NCORES)), trace=trace)
    except Exception:
        if not trace:
            raise
        res = run_bass_kernel_spmd(nc, in_maps, list(range(NCORES)), trace=False)
    LAST_EXEC_TIME_NS = res.exec_time_ns
    LAST_PROFILE = res.profile_json
    outs = [
        np.asarray(res.results[c]["out"]).T.reshape(BC, 3, 3, 100)
        for c in range(NCORES)
    ]
    return np.concatenate(outs, axis=0)


# revision 21
# speedup vs baseline: 1.2551x; 1.2551x over previous
"""BagNet Trainium2 Bass kernel, 8-core data parallel — v4/v5 layouts.

Cost model: matmul time = N (streamed output columns) x 0.42ns, independent
of K and M.  v3 streamed ~1.13M rows; v4/v5 restructures to ~716K:

  - conv1 ki-fold (v4): input stored as 3 y-shifted replicas stacked on the
    K axis (xt3 [84=(ky,x), p, y']), so each x-block h needs ONE K=84 matmul
    instead of 3 K=28 matmuls.  39 -> 13 matmuls/group (146K -> 49K rows).
  - conv2 parity-merge (v4): M=128=(xo-parity, co) instead of M=64=co.  Each
    matmul's lhsT covers 3 of the 9 taps (zero quadrant = 1/4 waste); 2
    matmuls per ki x 3 ki = 6 K=128/M=128 matmuls per patch cover everything
    (was 12 at M=64).  497K -> 249K rows.
  - conv3 D-repair (v5): the 6 per-patch K=64 single-tap matmuls are
    re-paired into K=128 by building y-shifted copies (D_e/D_o) of the A2
    dxo-planes via SBUF->SBUF DMA.  12 -> 10 matmuls/patch-pair
    (418K -> 348K rows).

Evac (PSUM->SBUF relu+bias) on alternating Scalar/Vector writes contiguous
destinations; wf streamed + reordered host-side to h3's (par, xg, yo)
s-order (unchanged from v3).
"""
import os
import numpy as np
import ml_dtypes
from contextlib import ExitStack

import concourse.bass as bass
import concourse.tile as tile
from concourse import bacc, mybir
from concourse.bass_utils import run_bass_kernel_spmd

LAST_EXEC_TIME_NS = None
LAST_PROFILE = None

EPS = 1e-5
NCORES = 8
BC = 16          # images per core
P = BC * 9       # patches per core
G = 16           # patches per pipeline group
NG = P // G
CH_S = int(os.environ.get("KN_CHS", "22"))   # wf s-slices per DMA chunk
NCH = 484 // CH_S
assert CH_S * NCH == 484
DT = mybir.dt.bfloat16
NPDT = ml_dtypes.bfloat16
F32 = mybir.dt.float32
RELU = mybir.ActivationFunctionType.Relu
IDENT = mybir.ActivationFunctionType.Identity
ADD = mybir.AluOpType.add
MAX = mybir.AluOpType.max

EV1 = os.environ.get("KN_EV1", "sv")
EV2 = os.environ.get("KN_EV2", "sv")
EV3 = os.environ.get("KN_EV3", "sv")
PSBUFS = int(os.environ.get("KN_PSBUFS", "7"))
WFB = int(os.environ.get("KN_WFB", "3"))
USE_D = bool(int(os.environ.get("KN_D", "0")))   # conv3 D-repair vs singles
WF2R = bool(int(os.environ.get("KN_WF2R", "0")))  # wf on both HW-DGE rings
PH = frozenset(os.environ.get("KN_PH", "c1,c2,c3,fc").split(","))


def build_program(repeat=1):
    nc = bacc.Bacc("TRN2", target_bir_lowering=False, debug=False)

    xt_d = nc.dram_tensor("xt3", [84, P * 26], DT, kind="ExternalInput")
    w1_d = nc.dram_tensor("w1k", [84, 13 * 128], DT, kind="ExternalInput")
    w2_d = nc.dram_tensor("w2m", [128, 6 * 128], DT, kind="ExternalInput")
    w3p_d = nc.dram_tensor("w3p", [128, 6 * 128], DT, kind="ExternalInput")
    w3s_d = nc.dram_tensor("w3s", [128, 3 * 128], DT, kind="ExternalInput")
    w3d_d = nc.dram_tensor("w3d", [128, 4 * 128], DT, kind="ExternalInput")
    b1_d = nc.dram_tensor("b1v", [128, 1], F32, kind="ExternalInput")
    b2_d = nc.dram_tensor("b2v", [128, 1], F32, kind="ExternalInput")
    b3_d = nc.dram_tensor("b3v", [128, 1], F32, kind="ExternalInput")
    bf_d = nc.dram_tensor("bfv", [100, 1], F32, kind="ExternalInput")
    # wfq laid out so each FC chunk is one fully-contiguous DRAM block:
    # row block c*128..(c+1)*128 holds chunk c as [128, CH_S*100]
    wf_d = nc.dram_tensor("wfq", [NCH * 128, CH_S * 100], DT,
                          kind="ExternalInput")
    out_d = nc.dram_tensor("out", [100, P], F32, kind="ExternalOutput")

    def evac(mode, idx, dst, src, bias):
        """relu(src + bias) -> dst, engine rotation string over
        ScalarE ('s'), VectorE ('v'); 'h' splits each evac by partition
        halves across both engines (halves per-bank latency)."""
        eng = mode[idx % len(mode)]
        if eng == "h":
            a, b = (0, 64) if idx % 2 == 0 else (64, 0)
            nc.scalar.activation(dst[a:a + 64], src[a:a + 64], RELU,
                                 bias=bias[a:a + 64], scale=1.0)
            nc.vector.tensor_scalar(dst[b:b + 64], src[b:b + 64],
                                    bias[b:b + 64], 0.0, ADD, MAX)
        elif eng == "s":
            nc.scalar.activation(dst, src, RELU, bias=bias, scale=1.0)
        else:
            nc.vector.tensor_scalar(dst, src, bias, 0.0, ADD, MAX)

    with tile.TileContext(nc) as tc, ExitStack() as ctx:
        const = ctx.enter_context(tc.tile_pool(name="const", bufs=1))
        a1p = ctx.enter_context(
            tc.tile_pool(name="a1", bufs=int(os.environ.get("KN_A1B", "1"))))
        a2p = ctx.enter_context(tc.tile_pool(name="a2", bufs=1))
        dp = ctx.enter_context(tc.tile_pool(name="dd", bufs=1))
        h3p = ctx.enter_context(tc.tile_pool(name="h3", bufs=1))
        wfp = ctx.enter_context(tc.tile_pool(name="wf", bufs=WFB))
        ps = ctx.enter_context(tc.tile_pool(name="ps", bufs=PSBUFS, space="PSUM"))
        fcp = ctx.enter_context(tc.tile_pool(name="fcps", bufs=1, space="PSUM"))

        xt3 = const.tile([84, P, 26], DT)
        nc.sync.dma_start(xt3[:], xt_d[:].rearrange("a (p y) -> a p y", y=26))
        w1k = const.tile([84, 13 * 128], DT)
        nc.sync.dma_start(w1k[:], w1_d[:])
        w2m = const.tile([128, 6 * 128], DT)
        nc.sync.dma_start(w2m[:], w2_d[:])
        w3pr = const.tile([128, 6 * 128], DT)
        nc.sync.dma_start(w3pr[:], w3p_d[:])
        w3sg = const.tile([128, 3 * 128], DT)
        nc.sync.dma_start(w3sg[:], w3s_d[:])
        w3d = const.tile([128, 4 * 128], DT)
        nc.sync.dma_start(w3d[:], w3d_d[:])
        b1t = const.tile([128, 1], F32)
        nc.sync.dma_start(b1t[:], b1_d[:])
        b2t = const.tile([128, 1], F32)
        nc.sync.dma_start(b2t[:], b2_d[:])
        b3t = const.tile([128, 1], F32)
        nc.sync.dma_start(b3t[:], b3_d[:])
        bft = const.tile([100, 1], F32)
        nc.sync.dma_start(bft[:], bf_d[:])

        # h3 is s-major so the FC matmul rhs columns are CONTIGUOUS
        # (a patch-major h3 makes the PE stream 2-byte strided columns,
        # which measured ~4x slower than contiguous). The conv3 evacs pay
        # with strided writes instead, hidden under the tensor-bound convs.
        h3 = h3p.tile([128, 484, P], DT)
        fc_psum = fcp.tile([128, P], F32)

        rep_ctx = tc.For_i(0, repeat, 1) if repeat > 1 else None
        if rep_ctx is not None:
            rep_ctx.__enter__()
        # pre-issue the first WFB wf chunks so they stream during the convs;
        # alternate the two physical HW-DGE rings (SP / Activation issued)
        wf_eng = [nc.sync, nc.scalar] if WF2R else [nc.sync, nc.sync]
        wf_pre = []
        for c in range(min(WFB, NCH) if "fc" in PH else 0):
            wfb = wfp.tile([128, CH_S * 100], DT)
            wf_eng[c % 2].dma_start(wfb[:], wf_d[c * 128:(c + 1) * 128, :])
            wf_pre.append(wfb)
        for g in range(NG if "c1" in PH else 0):
            A1x = a1p.tile([128, 13, G, 26], DT)
            A2 = a2p.tile([128, G, 12, 24], DT)
            if USE_D:
                D_e = dp.tile([128, G, 12, 24], DT)
                D_o = dp.tile([128, G, 12, 24], DT)
                A2f = A2[:, :, :, :].rearrange("q g n y -> q (g n y)")
                D_ef = D_e[:, :, :, :].rearrange("q g n y -> q (g n y)")
                D_of = D_o[:, :, :, :].rearrange("q g n y -> q (g n y)")

            # ---- conv1: K=84=(ky,x) ki-folded; 13 matmuls of N=416 ----
            for h in range(13):
                c1 = ps.tile([128, G * 26], F32, tag="cps")
                nc.tensor.matmul(
                    c1[:, :], w1k[0:84, h * 128:(h + 1) * 128],
                    xt3[0:84, g * G:(g + 1) * G, 0:26],
                    start=True, stop=True)
                evac(EV1, h, A1x[:, h, :, :], c1[:, :], b1t[:, :])

            # ---- conv2: parity-merged M=128=(par,co); 6 matmuls/patch ----
            for pl in range(G if "c2" in PH else 0):
                c2 = ps.tile([128, 12, 24], F32, tag="cps")
                for ki in range(3):
                    nc.tensor.matmul(
                        c2[:], w2m[:, (2 * ki) * 128:(2 * ki + 1) * 128],
                        A1x[:, 0:12, pl, ki:ki + 24],
                        start=(ki == 0), stop=False)
                    nc.tensor.matmul(
                        c2[:], w2m[:, (2 * ki + 1) * 128:(2 * ki + 2) * 128],
                        A1x[:, 1:13, pl, ki:ki + 24],
                        start=False, stop=(ki == 2))
                evac(EV2, pl, A2[:, pl, :, :], c2[:], b2t[:, :])
                if USE_D and pl % 2 == 1 and "c3" in PH:
                    # flat-offset y-shift copies of the A2 dxo-planes for this
                    # patch pair: band0 = plane, band1 = plane shifted +1 elem
                    # (wrapped slots land at y=23, never read by the matmuls)
                    b0 = (pl - 1) * 288
                    nc.sync.dma_start(
                        D_ef[0:64, b0:b0 + 576], A2f[0:64, b0:b0 + 576])
                    nc.sync.dma_start(
                        D_ef[64:128, b0:b0 + 575], A2f[0:64, b0 + 1:b0 + 576])
                    nc.sync.dma_start(
                        D_of[0:64, b0:b0 + 576], A2f[64:128, b0:b0 + 576])
                    nc.sync.dma_start(
                        D_of[64:128, b0:b0 + 575], A2f[64:128, b0 + 1:b0 + 576])

            # ---- conv3: pairs + (D-repair | K=64 singles) ----
            if "c3" not in PH:
                continue
            if USE_D:
                for pq in range(G // 2):
                    pl = 2 * pq
                    pv = slice(pl, pl + 2)
                    c3e = ps.tile([128, 2, 11, 22], F32, tag="cps")
                    c3o = ps.tile([128, 2, 11, 22], F32, tag="cps")
                    for ki in range(3):
                        yv = slice(ki, ki + 22)
                        nc.tensor.matmul(
                            c3e[:], w3pr[:, ki * 128:(ki + 1) * 128],
                            A2[:, pv, 0:11, yv], start=(ki == 0), stop=False)
                        nc.tensor.matmul(
                            c3o[:], w3pr[:, (3 + ki) * 128:(4 + ki) * 128],
                            A2[:, pv, 1:12, yv], start=(ki == 0), stop=False)
                    nc.tensor.matmul(  # taps (kh 0+1, kw2) via D_e bands
                        c3e[:], w3d[:, 0:128],
                        D_e[:, pv, 1:12, 0:22], start=False, stop=False)
                    nc.tensor.matmul(  # tap (kh2, kw2) direct from A2
                        c3e[:], w3d[0:64, 128:256],
                        A2[0:64, pv, 1:12, 2:24], start=False, stop=True)
                    nc.tensor.matmul(  # taps (kh 0+1, kw0) via D_o bands
                        c3o[:], w3d[:, 2 * 128:3 * 128],
                        D_o[:, pv, 0:11, 0:22], start=False, stop=False)
                    nc.tensor.matmul(  # tap (kh2, kw0) direct from A2
                        c3o[:], w3d[64:128, 3 * 128:4 * 128],
                        A2[64:128, pv, 0:11, 2:24], start=False, stop=True)
                    pg = g * G + pl
                    evac(EV3, pq,
                         h3[:, 0:242, pg:pg + 2].rearrange("q s p -> q p s"),
                         c3e[:], b3t[:, :])
                    evac(EV3, pq + 1,
                         h3[:, 242:484, pg:pg + 2].rearrange("q s p -> q p s"),
                         c3o[:], b3t[:, :])
            else:
                for pq in range(G // 2):
                    pl = 2 * pq
                    pv = slice(pl, pl + 2)
                    c3e = ps.tile([128, 2, 11, 22], F32, tag="cps")
                    c3o = ps.tile([128, 2, 11, 22], F32, tag="cps")
                    for ki in range(3):
                        yv = slice(ki, ki + 22)
                        nc.tensor.matmul(
                            c3e[:], w3pr[:, ki * 128:(ki + 1) * 128],
                            A2[:, pv, 0:11, yv], start=(ki == 0), stop=False)
                        nc.tensor.matmul(
                            c3o[:], w3pr[:, (3 + ki) * 128:(4 + ki) * 128],
                            A2[:, pv, 1:12, yv], start=(ki == 0), stop=False)
                        nc.tensor.matmul(
                            c3e[:], w3sg[0:64, ki * 128:(ki + 1) * 128],
                            A2[0:64, pv, 1:12, yv], start=False, stop=(ki == 2))
                        nc.tensor.matmul(
                            c3o[:], w3sg[64:128, ki * 128:(ki + 1) * 128],
                            A2[64:128, pv, 0:11, yv], start=False, stop=(ki == 2))
                    pg = g * G + pl
                    evac(EV3, pq,
                         h3[:, 0:242, pg:pg + 2].rearrange("q s p -> q p s"),
                         c3e[:], b3t[:, :])
                    evac(EV3, pq + 1,
                         h3[:, 242:484, pg:pg + 2].rearrange("q s p -> q p s"),
                         c3o[:], b3t[:, :])

        # ---- FC: 484 accumulating K=128 matmuls, wf streamed ----
        if "fc" not in PH:
            nc.vector.memset(fc_psum[0:100, :], 0.0)
        if "fc" in PH and "c3" not in PH:
            nc.vector.memset(h3[:], 0.0)
        for c in range(NCH if "fc" in PH else 0):
            if c < len(wf_pre):
                wfb = wf_pre[c]
            else:
                wfb = wfp.tile([128, CH_S * 100], DT)
                wf_eng[c % 2].dma_start(
                    wfb[:], wf_d[c * 128:(c + 1) * 128, :])
            for sl in range(CH_S):
                s = c * CH_S + sl
                nc.tensor.matmul(
                    fc_psum[0:100, :], wfb[:, sl * 100:(sl + 1) * 100],
                    h3[:, s, :],
                    start=(s == 0), stop=(s == 483))
        outb = const.tile([100, P], F32)
        nc.scalar.activation(outb[:], fc_psum[0:100, :], IDENT,
                             bias=bft[:, :], scale=1.0)
        nc.sync.dma_start(out_d[:], outb[:])
        if rep_ctx is not None:
            rep_ctx.__exit__(None, None, None)

    nc.compile()
    return nc


def _fold_bn(w, b, g_, be, m, v):
    s = (g_ / np.sqrt(v + EPS)).astype(np.float32)
    return (w * s[:, None, None, None]).astype(np.float32), \
           (be - (m - b) * s).astype(np.float32)


def prep_shared(inputs):
    """Host-side: fold BN, build weight layouts shared by all cores."""
    f = np.float32
    w1f, b1f = _fold_bn(inputs["w1"], inputs["b1"], inputs["g1"], inputs["be1"],
                        inputs["m1"], inputs["v1"])
    w2f, b2f = _fold_bn(inputs["w2"], inputs["b2"], inputs["g2"], inputs["be2"],
                        inputs["m2"], inputs["v2"])
    w3f, b3f = _fold_bn(inputs["w3"], inputs["b3"], inputs["g3"], inputs["be3"],
                        inputs["m3"], inputs["v3"])

    # conv1 ki-folded lhsT: w1k[(ky,x), (dx,co)] per h-block
    w1r = w1f[:, 0]                      # (64, kh, kw)
    w1k = np.zeros((84, 13 * 128), f)
    for h in range(13):
        for r in range(3):
            for dx in range(2):
                for kx in range(3):
                    xx = 2 * h + dx + kx
                    w1k[28 * r + xx,
                        h * 128 + dx * 64:h * 128 + dx * 64 + 64] = w1r[:, r, kx]

    # conv2 parity-merged lhsT blocks (ki, slot): [128=(dx,ci), 128=(par,co)]
    w2m = np.zeros((128, 6 * 128), f)
    for ki in range(3):
        b0 = (2 * ki) * 128
        b1 = (2 * ki + 1) * 128
        w2m[0:64, b0:b0 + 64] = w2f[:, :, ki, 0].T       # even, kw0 @ dx0
        w2m[64:128, b0:b0 + 64] = w2f[:, :, ki, 1].T     # even, kw1 @ dx1
        w2m[64:128, b0 + 64:b0 + 128] = w2f[:, :, ki, 0].T  # odd, kw0 @ dx1
        w2m[0:64, b1:b1 + 64] = w2f[:, :, ki, 2].T       # even, kw2 @ dx0(+1)
        w2m[0:64, b1 + 64:b1 + 128] = w2f[:, :, ki, 1].T    # odd, kw1 @ dx0(+1)
        w2m[64:128, b1 + 64:b1 + 128] = w2f[:, :, ki, 2].T  # odd, kw2 @ dx1(+1)

    def tap_tiles(wt, co):  # wt (co, 64, 3, 3)
        pr = np.zeros((128, 6 * co), f)
        sg = np.zeros((128, 3 * co), f)
        for ki in range(3):
            pr[0:64, ki * co:(ki + 1) * co] = wt[:, :, ki, 0].T
            pr[64:128, ki * co:(ki + 1) * co] = wt[:, :, ki, 1].T
            pr[0:64, (3 + ki) * co:(4 + ki) * co] = wt[:, :, ki, 1].T
            pr[64:128, (3 + ki) * co:(4 + ki) * co] = wt[:, :, ki, 2].T
            sg[0:64, ki * co:(ki + 1) * co] = wt[:, :, ki, 2].T
            sg[64:128, ki * co:(ki + 1) * co] = wt[:, :, ki, 0].T
        return pr, sg

    w3p, w3s = tap_tiles(w3f, 128)

    # conv3 D-weights: blocks [De-pair, De-single, Do-pair, Do-single].
    # The Do-single reads A2[64:128] directly, so its lhsT lives at
    # partitions 64:128 (matmul requires matching base partitions).
    w3d = np.zeros((128, 4 * 128), f)
    for r in range(2):
        w3d[r * 64:(r + 1) * 64, 0:128] = w3f[:, :, r, 2].T
        w3d[r * 64:(r + 1) * 64, 2 * 128:3 * 128] = w3f[:, :, r, 0].T
    w3d[0:64, 128:256] = w3f[:, :, 2, 2].T
    w3d[64:128, 3 * 128:4 * 128] = w3f[:, :, 2, 0].T

    # wf reordered to h3's (par, xg, yo) s-ordering
    wfr = inputs["wf"].astype(f).reshape(100, 128, 22, 22)   # (out, co, yo, xo)
    wfr = wfr.transpose(0, 1, 3, 2)                          # (out, co, xo, yo)
    wfr = wfr.reshape(100, 128, 11, 2, 22).transpose(0, 1, 3, 2, 4)
    wfq = np.ascontiguousarray(
        wfr.reshape(100, 128, 484).transpose(1, 2, 0)
    ).reshape(128, 484 * 100)
    # chunk-contiguous DRAM layout: [NCH*128, CH_S*100], chunk c in rows
    # c*128..(c+1)*128
    wfq = np.ascontiguousarray(
        wfq.reshape(128, NCH, CH_S * 100).transpose(1, 0, 2)
    ).reshape(NCH * 128, CH_S * 100)

    return {
        "w1k": w1k.astype(NPDT),
        "w2m": w2m.astype(NPDT),
        "w3p": w3p.astype(NPDT), "w3s": w3s.astype(NPDT),
        "w3d": w3d.astype(NPDT),
        "b1v": np.tile(b1f, 2)[:, None].astype(f),
        "b2v": np.tile(b2f, 2)[:, None].astype(f),
        "b3v": b3f[:, None].astype(f),
        "bfv": inputs["bf"].astype(f)[:, None],
        "wfq": wfq.astype(NPDT),
    }


def prep_core(x, c):
    """Per-core input: x-pixel-on-partition patch layout, 3 y-shifted bands."""
    xs = np.asarray(x)[c * BC:(c + 1) * BC, 0].astype(np.float32)  # (16,84,84)
    arr = xs.reshape(BC, 3, 28, 3, 28).transpose(4, 0, 1, 3, 2)    # (x,b,Hb,Wb,y)
    arr = arr.reshape(28, P, 28)
    xt3 = np.zeros((84, P, 26), np.float32)
    for r in range(3):
        xt3[28 * r:28 * r + 28] = arr[:, :, r:r + 26]
    return {"xt3": xt3.reshape(84, P * 26).astype(NPDT)}


def _make_runner(nc, in_maps):
    import jax
    from jax.sharding import Mesh, PartitionSpec, NamedSharding
    from jax.experimental.shard_map import shard_map
    from concourse import mybir as _mb
    from concourse import bass2jax

    bass2jax.install_neuronx_cc_hook()
    partition_name = nc.partition_id_tensor.name if nc.partition_id_tensor else None
    in_names, out_names, out_avals, zero_outs = [], [], [], []
    for alloc in nc.m.functions[0].allocations:
        if not isinstance(alloc, _mb.MemoryLocationSet):
            continue
        name = alloc.memorylocations[0].name
        if alloc.kind == "ExternalInput":
            if name != partition_name:
                in_names.append(name)
        elif alloc.kind == "ExternalOutput":
            shape = tuple(alloc.tensor_shape)
            dtype = _mb.dt.np(alloc.dtype)
            out_names.append(name)
            out_avals.append(jax.core.ShapedArray(shape, dtype))
            zero_outs.append(np.zeros(shape, dtype))
    n_params = len(in_names)
    all_names = in_names + out_names
    if partition_name is not None:
        all_names = all_names + [partition_name]
    donate = tuple(range(n_params, n_params + len(out_names)))

    def _body(*args):
        operands = list(args)
        if partition_name is not None:
            operands.append(bass2jax.partition_id_tensor())
        outs = bass2jax._bass_exec_p.bind(
            *operands,
            out_avals=tuple(out_avals),
            in_names=tuple(all_names),
            out_names=tuple(out_names),
            lowering_input_output_aliases=(),
            sim_require_finite=True,
            sim_require_nnan=True,
            nc=nc,
        )
        return tuple(outs)

    devices = jax.devices()[:NCORES]
    mesh = Mesh(np.asarray(devices), ("core",))
    spec = NamedSharding(mesh, PartitionSpec("core"))
    sharded = jax.jit(
        shard_map(_body, mesh=mesh,
                  in_specs=(PartitionSpec("core"),) * (n_params + len(out_names)),
                  out_specs=(PartitionSpec("core"),) * len(out_names),
                  check_rep=False),
        donate_argnums=donate, keep_unused=True)

    concat_in = [
        jax.device_put(
            np.concatenate([np.asarray(in_maps[c][n]) for c in range(NCORES)],
                           axis=0), spec)
        for n in in_names
    ]

    def _zeros():
        return [jax.device_put(
            np.zeros((NCORES * z.shape[0], *z.shape[1:]), z.dtype), spec)
            for z in zero_outs]

    def dispatch():
        import time as _time
        zs = _zeros()
        jax.block_until_ready(zs)
        t0 = _time.perf_counter()
        r = sharded(*concat_in, *zs)
        jax.block_until_ready(r)
        return _time.perf_counter() - t0

    r = sharded(*concat_in, *_zeros())   # compile + warm
    jax.block_until_ready(r)
    return dispatch


def bench(inputs, iters=14, repeat=128):
    """Per-iteration HW time: interleaved dispatches of the repeat=1 and
    repeat=`repeat` programs; median difference resists the bimodal axon
    dispatch-overhead noise."""
    inputs = {k: np.asarray(v) for k, v in inputs.items()}
    shared = prep_shared(inputs)
    in_maps = [{**shared, **prep_core(inputs["x"], c)} for c in range(NCORES)]
    d1 = _make_runner(build_program(repeat=1), in_maps)
    dr = _make_runner(build_program(repeat=repeat), in_maps)
    t1s, trs = [], []
    for _ in range(iters):
        t1s.append(d1())
        trs.append(dr())
    t1s.sort()
    trs.sort()
    med1 = t1s[len(t1s) // 2]
    medr = trs[len(trs) // 2]
    med = (medr - med1) * 1e9 / (repeat - 1)
    mn = (trs[0] - t1s[0]) * 1e9 / (repeat - 1)
    print(f"[bench] med {med:.0f} ns  min {mn:.0f} ns  "
          f"t1 {[f'{t*1e3:.1f}' for t in t1s]} "
          f"tr {[f'{t*1e3:.1f}' for t in trs]}")
    return med


def kernel(**inputs):
    global LAST_EXEC_TIME_NS, LAST_PROFILE
    inputs = {k: np.asarray(v) for k, v in inputs.items()}
    shared = prep_shared(inputs)
    in_maps = [{**shared, **prep_core(inputs["x"], c)} for c in range(NCORES)]
    nc = build_program()
    trace = bool(os.environ.get("BASS_KERNEL_TRACE"))
    try:
        res = run_bass_kernel_spmd(nc, in_maps, list(range(NCORES)), trace=trace)
    except Exception:
        if not trace:
            raise
        res = run_bass_kernel_spmd(nc, in_maps, list(range(NCORES)), trace=False)
    LAST_EXEC_TIME_NS = res.exec_time_ns
    LAST_PROFILE = res.profile_json
    outs = [
        np.asarray(res.results[c]["out"]).T.reshape(BC, 3, 3, 100)
        for c in range(NCORES)
    ]
    return np.concatenate(outs, axis=0)
